# revision 29
# baseline (speedup 1.0000x reference)
"""AttentionBlock kernel for 8 Trainium2 NeuronCores (Bass/Tile), fp8 edition.

Problem (hardcoded shapes): x [16, 512, 32, 32] fp32, GroupNorm(32 groups,
eps=1e-5) -> 1x1-conv QKV (qkv_w [1536,512], qkv_b) -> 8-head attention over
T=1024 positions (head dim 64) -> 1x1-conv proj -> residual add.

Sharding: pure data-parallel over batch; each of the 8 cores handles 2
batches end-to-end; weights replicated; no collectives.

All five matmul families (QKV, v^T, St, AV, proj) run in fp8e4m3 with
MatmulPerfMode.DoubleRow: both operands packed [K, 2, .], one instruction
contracts 256 rows at the PE's fixed 1 output row/cycle (2x bf16 work per
instruction for the contraction-bound families; St, which is output-bound,
costs the same as bf16 but the spare DR slice carries a constant).

Scaling scheme (fp8 e4m3 here: bias 7, max 240, bits 120+ are inf):
  q' = 8q, k' = 8k, v' = 8v carried in fp8 (weights pre-scaled x8 on host;
  Wq an extra x8 so its fp8 entries stay normal-range, undone in the q
  psum->sbuf copy).  St psum = q'.k' = 64*logit + BETA (see below).  The
  proj matmul computes 64*proj(a) + 64*bp -- the proj bias rides a third
  DoubleRow step whose lhsT is a constant pattern and whose rhs is the q2
  ones slice -- all undone by the 1/64 in the residual
  scalar_tensor_tensor.

St DoubleRow slices: slice0 = zero-padded k' / q' data; slice1 = a rank-1
constant (BETA=160 at partition 0 of kz2, 1.0 at partition 0 of q2) adding
BETA to every logit inside the matmul.  That makes exp-as-fp8-bits a single
op: fp8 bits of exp((P-BETA)/64 - S) equal P*0.18034 for this BETA, so one
DVE tensor_scalar (mult, max-with-0) -> uint8 convert writes the attention
weights directly as fp8 bits (Schraudolph).  The other share of tiles uses
the real ACT exp table (scale=1/64, bias=-S-BETA/64); the two es families
agree up to softmax-invariant constants.  sc-pairs split engines so each
AV DoubleRow rhs pair is ready in one exp latency.  GPSIMD cannot touch
PSUM on this HW (and its TensorScalarPtr path is ~17x slow), so the
softmax + all psum->sbuf drains live on ACT+DVE only; GPSIMD gets
SBUF-only work (a-mults, reciprocal seeds, h-norm stays DVE).

Denominators come free (ones columns in the AV lhsT, replicated on the
dead 64 partitions), one magic-seed+Newton reciprocal chain per HEAD PAIR
(so only the last pair's chain sits on the tail), broadcast via a DRAM
bounce mid-run; the tail pair instead broadcasts 1/D across partitions
with a tiny PE matmul into PSUM (saves three serial DMA hops) and its
a-mults run on DVE.  proj(b-1) is cut into per-(m,half) units drained two
per head inside batch b's attention to fill the exp-gated PE gaps; the
last batch's proj j=0 group joins that queue after pair 1's chain.
GroupNorm runs in f32 (bn_stats + tiny constant matmuls, rsqrt via Newton
so ACT keeps only the exp table loaded).  Large constant patterns (kz2
pads/beta, q2 ones) are DMA-broadcast from a small host tensor instead of
memset (engine memsets are expensive; inputs stay slim for the load
barrier).
"""

import numpy as np

B, C, T = 16, 512, 1024
NH, CH = 8, 64
NG = 32
EPS = 1e-5
NCORES = 8
BPC = B // NCORES  # batches per core
KO = C // 128      # channel chunks

TRACE = False
DEBUG_LIGHT = False  # only h + a outputs (minimal schedule perturbation)

# fp8 softmax constants (see module docstring)
BETA = 160.0
A_CONST = float(8 * np.log2(np.e) / 64.0)          # 0.18033688
S_EFF = float((56.5 - 8 * np.log2(np.e) * (BETA / 64.0)) / (8 * np.log2(np.e)))
B_ACT = float(-S_EFF - BETA / 64.0)

# exp engine routing (A=ACT exp table, D=DVE Schraudolph); GPSIMD cannot
# access PSUM on this HW, so softmax drains only through ACT/DVE.  N_ACT_EXP
# of the 128 tiles (2 batches x 64) go to ACT, evenly spread.
N_EXP = 2 * NH * 8
N_ACT_EXP = 72


def _exp_on_act(idx):
    # each sc-pair's two exps go to different engines so the pair is ready
    # in one exp-latency; extra ACT share on alternate heads (~56% ACT)
    h, sc = (idx // 8) % NH, idx % 8
    if sc == 7 and h % 2 == 0:
        return True
    return sc % 2 == 0


def _np8():
    import ml_dtypes
    return np.dtype(ml_dtypes.float8_e4m3)


def _npbf():
    import ml_dtypes
    return np.dtype(ml_dtypes.bfloat16)


def _build_nc():
    import concourse.bass as bass
    import concourse.tile as tile
    from concourse import bacc, mybir
    from contextlib import ExitStack

    f32 = mybir.dt.float32
    bf16 = mybir.dt.bfloat16
    fp8 = mybir.dt.float8e4
    u8 = mybir.dt.uint8
    i32 = mybir.dt.int32
    DR = mybir.MatmulPerfMode.DoubleRow

    nc = bacc.Bacc()
    AF = mybir.ActivationFunctionType
    ALU = mybir.AluOpType

    x_d = nc.dram_tensor("x", [BPC, 128, KO, T], bf16, kind="ExternalInput")
    wqk_d = nc.dram_tensor("wqkT", [128, KO, 2 * C], fp8, kind="ExternalInput")
    wv_d = nc.dram_tensor("wvT", [128, KO, C], fp8, kind="ExternalInput")
    wp_d = nc.dram_tensor("wpT", [128, KO, C], fp8, kind="ExternalInput")
    bq_d = nc.dram_tensor("bq", [128, KO], f32, kind="ExternalInput")
    bpp_d = nc.dram_tensor("bppat", [128, 2, C], fp8, kind="ExternalInput")
    g_d = nc.dram_tensor("gmat", [128, KO, NG], f32, kind="ExternalInput")
    b_d = nc.dram_tensor("bmat", [128, KO, 128], f32, kind="ExternalInput")
    ones_d = nc.dram_tensor("ones", [128, 64], fp8, kind="ExternalInput")
    rbc_d = nc.dram_tensor("rbc", [128, 2, 128], bf16, kind="ExternalInput")
    # pat: cols 0:T beta-at-partition0, T:2T ones-at-partition0, 2T:3T zeros
    pat_d = nc.dram_tensor("pat", [128, 3 * T], fp8, kind="ExternalInput")
    out_d = nc.dram_tensor("out", [BPC, 128, KO, T], bf16, kind="ExternalOutput")
    if DEBUG_LIGHT:
        dbg_h = nc.dram_tensor("dbg_h", [BPC, 128, KO, T], fp8, kind="ExternalOutput")
        dbg_a = nc.dram_tensor("dbg_a", [BPC, 128, KO, T], fp8, kind="ExternalOutput")

    # Every matmul keeps the PE in the default 128-row tiling mode.
    def mm(out, lhsT, rhs, **kw):
        assert lhsT.partition_size() == 128
        return nc.tensor.matmul(out, lhsT, rhs, **kw)

    with tile.TileContext(nc) as tc, ExitStack() as ctx:
        consts = ctx.enter_context(tc.tile_pool(name="consts", bufs=1))
        xp = ctx.enter_context(tc.tile_pool(name="xp", bufs=2))
        hp = ctx.enter_context(tc.tile_pool(name="hp", bufs=2))
        qp = ctx.enter_context(tc.tile_pool(name="qp", bufs=2))
        kzp = ctx.enter_context(tc.tile_pool(name="kzp", bufs=1))
        ksp = ctx.enter_context(tc.tile_pool(name="ksp", bufs=2))
        vtp = ctx.enter_context(tc.tile_pool(name="vtp", bufs=1))
        esp = ctx.enter_context(tc.tile_pool(name="esp", bufs=12))
        avsp = ctx.enter_context(tc.tile_pool(name="avsp", bufs=6))
        rbp = ctx.enter_context(tc.tile_pool(name="rbp", bufs=2))
        dcp = ctx.enter_context(tc.tile_pool(name="dcp", bufs=2))
        yp = ctx.enter_context(tc.tile_pool(name="yp", bufs=2))
        ap_ = ctx.enter_context(tc.tile_pool(name="ap", bufs=2))
        gnp = ctx.enter_context(tc.tile_pool(name="gnp", bufs=2))
        psS = ctx.enter_context(tc.tile_pool(name="psS", bufs=2, space="PSUM"))
        psB = ctx.enter_context(tc.tile_pool(name="psB", bufs=1, space="PSUM"))
        psQ = ctx.enter_context(tc.tile_pool(name="psQ", bufs=2, space="PSUM"))
        rdp = ctx.enter_context(tc.tile_pool(name="rdp", bufs=4, space="DRAM"))

        # ---- small constants first (GroupNorm needs only these + x) ----
        bq_sb = consts.tile([128, KO], f32)
        nc.sync.dma_start(bq_sb[:], bq_d[:])
        bpp_sb = consts.tile([128, 2, C], fp8)
        nc.sync.dma_start(bpp_sb[:], bpp_d[:])
        g_sb = consts.tile([128, KO, NG], f32)
        nc.sync.dma_start(g_sb[:], g_d[:])
        bm_sb = consts.tile([128, KO, 128], f32)
        # ACT exp scale/bias scalars
        bact_sb = consts.tile([128, 2], f32)
        nc.gpsimd.memset(bact_sb[:, 0:1], B_ACT)
        nc.gpsimd.memset(bact_sb[:, 1:2], 1.0 / 64.0)
        # Magic seed constant for the Newton reciprocal (fast-inverse trick).
        magic_sb = consts.tile([128, 2], i32)
        nc.gpsimd.memset(magic_sb[:], 0x7EF127EA)
        # Magic seed for Newton rsqrt.
        magic_rs = consts.tile([NG, 1], i32)
        nc.gpsimd.memset(magic_rs[:], 0x5F3759DF)
        # tail reciprocal-broadcast: lhsT patterns + a 128-partition y tile
        # (rows 4:128 zeroed once; matmul rhs must be garbage-free)
        rbc_sb = consts.tile([128, 2, 128], bf16)
        nc.sync.dma_start(rbc_sb[:], rbc_d[:])
        yt_sb = consts.tile([128, 512], bf16)
        nc.gpsimd.memset(yt_sb[:], 0.0)

        # ---- batch 0 input before the big weights ----
        def emit_x_load(b):
            x_sb = xp.tile([128, KO, T], bf16, tag="x")
            for ko in range(KO):
                nc.sync.dma_start(x_sb[:, ko, :], x_d[b, :, ko, :])
            return x_sb

        x_tiles = [None] * BPC
        x_tiles[0] = emit_x_load(0)
        nc.sync.dma_start(bm_sb[:], b_d[:])

        wqk_sb = consts.tile([128, KO, 2 * C], fp8)
        nc.sync.dma_start(wqk_sb[:], wqk_d[:])
        wv_sb = consts.tile([128, KO, C], fp8)
        nc.sync.dma_start(wv_sb[:], wv_d[:])
        wp_sb = consts.tile([128, KO, C], fp8)
        nc.sync.dma_start(wp_sb[:], wp_d[:])

        # kz2 [128, 2, NH, T]: slice0 = zero-padded k' per head (head h's k'
        # on partitions 64*(h%2)..+64, zeros elsewhere); slice1 = beta
        # pattern (beta at partition 0, zeros elsewhere).  DMA-initialized
        # (pads on one queue, beta slice on another).
        kz2 = kzp.tile([128, 2, NH, T], fp8, tag="kz")
        # slice1 = beta pattern broadcast over heads; slice0 pads = zeros
        nc.sync.dma_start(
            kz2[:, 1, :, :],
            bass.AP(tensor=pat_d, offset=0, ap=[[3 * T, 128], [0, NH], [1, T]]))
        nc.sync.dma_start(
            kz2[64:128, 0, 0:NH:2, :],
            bass.AP(tensor=pat_d, offset=64 * 3 * T + 2 * T,
                    ap=[[3 * T, 64], [0, NH // 2], [1, T]]))
        nc.sync.dma_start(
            kz2[0:64, 0, 1:NH:2, :],
            bass.AP(tensor=pat_d, offset=2 * T,
                    ap=[[3 * T, 64], [0, NH // 2], [1, T]]))

        # q2 per batch [128, 2, KO, T]: slice0 = q' data, slice1 = ones
        # pattern (1.0 at partition 0).
        q2s = []
        for b in range(BPC):
            q2 = qp.tile([128, 2, KO, T], fp8, tag="q", name=f"q{b}")
            nc.sync.dma_start(
                q2[:, 1, :, :],
                bass.AP(tensor=pat_d, offset=T,
                        ap=[[3 * T, 128], [0, KO], [1, T]]))
            q2s.append(q2)
        # v^T lhsT buffer: per head-pair p the 192 columns are
        # [vT_even(64) | ones(64) | vT_odd(64)]; head 2p uses cols 0:128 and
        # head 2p+1 uses cols 64:192.  Ones blocks DMA'd once.
        vt_sb = vtp.tile([128, 8, 4, 192], fp8, tag="vt")
        ones_src = bass.AP(tensor=ones_d, offset=0,
                           ap=[[64, 128], [0, 32], [1, 64]])
        vt_flat = vt_sb[:].rearrange("p a b w -> p (a b) w")
        nc.sync.dma_start(vt_flat[:, :, 64:128], ones_src)

        # PE p-state warmup: harmless matmuls on already-loaded constants
        # during the GroupNorm ramp (PE would otherwise idle and downclock,
        # making the first QKV/St matmuls run at 1.2GHz)
        warm_ps = psQ.tile([128, 512], f32, tag="q")
        for w in range(24):
            mm(warm_ps[:], wqk_sb[:, 0, 0:128], wqk_sb[:, 0, 0:512],
               start=(w == 0), stop=(w == 23))

        # per-batch live tiles
        h_tiles = [None] * BPC

        def emit_gn_qkv(b):
            """GroupNorm + QKV (q,k) + v^T for batch b."""
            x_sb = x_tiles[b]

            # ---------------- GroupNorm (f32, as bf16 kernel) ------------
            rhs3 = gnp.tile([128, KO, 3], f32, tag="rhs3")
            for ko in range(KO):
                stats = gnp.tile([128, 2, 6], f32, tag="stats")
                for j in range(2):
                    nc.vector.bn_stats(out=stats[:, j, :], in_=x_sb[:, ko, 512 * j:512 * (j + 1)])
                nc.vector.bn_aggr(out=rhs3[:, ko, 0:2], in_=stats[:])
                nc.vector.tensor_mul(rhs3[:, ko, 2:3], rhs3[:, ko, 0:1], rhs3[:, ko, 0:1])
            gps = psQ.tile([NG, 3], f32, tag="q")
            for ko in range(KO):
                mm(gps[:], g_sb[:, ko, :], rhs3[:, ko, :],
                   start=(ko == 0), stop=(ko == KO - 1))
            gq = gnp.tile([NG, 3], f32, tag="gq")
            nc.vector.tensor_copy(gq[:], gps[:])
            gtmp = gnp.tile([NG, 2], f32, tag="gtmp")
            gst2 = gnp.tile([128, 2], f32, tag="gst2")
            nc.vector.memset(gst2[:], 0.0)
            nc.vector.tensor_copy(gst2[0:NG, 0:1], gq[:, 0:1])
            nc.vector.tensor_add(gtmp[:, 0:1], gq[:, 1:2], gq[:, 2:3])
            nc.vector.tensor_mul(gtmp[:, 1:2], gq[:, 0:1], gq[:, 0:1])
            nc.vector.tensor_sub(gtmp[:, 0:1], gtmp[:, 0:1], gtmp[:, 1:2])
            vpe = gtmp[:, 0:1]
            nc.vector.tensor_scalar_add(vpe, vpe, EPS)
            rs = gnp.tile([NG, 3], f32, tag="rs")
            ry = rs[:, 0:1]
            ra = rs[:, 1:2]
            rb = rs[:, 2:3]
            nc.vector.tensor_scalar(
                out=ra.bitcast(i32), in0=vpe.bitcast(i32),
                scalar1=1, scalar2=None, op0=ALU.arith_shift_right)
            nc.vector.tensor_tensor(
                out=ry.bitcast(i32), in0=magic_rs[:],
                in1=ra.bitcast(i32), op=ALU.subtract)
            for _ in range(1):
                nc.vector.tensor_mul(ra, vpe, ry)
                nc.vector.tensor_mul(rb, ra, ry)
                nc.vector.scalar_tensor_tensor(
                    out=ra, in0=rb, scalar=3.0, in1=ry,
                    op0=ALU.subtract, op1=ALU.mult)
                nc.vector.tensor_scalar_mul(ry, ra, -0.5)
            nc.vector.tensor_copy(gst2[0:NG, 1:2], ry)
            bst_ps = psQ.tile([128, 2 * KO], f32, tag="q")
            for ko in range(KO):
                mm(bst_ps[:, 2 * ko:2 * ko + 2], bm_sb[:, ko, :], gst2[:],
                   start=True, stop=True)
            bst = gnp.tile([128, 2 * KO], f32, tag="bst_sb")
            nc.vector.tensor_copy(bst[:], bst_ps[:])
            nbst = gnp.tile([128, KO], f32, tag="nbst")
            if b == 0:
                # -mean*rstd for the ACT-side h-norm (bias term)
                nc.vector.tensor_tensor(
                    out=nbst[:], in0=bst[:, 0:2 * KO:2], in1=bst[:, 1:2 * KO:2],
                    op=ALU.mult)
                nc.vector.tensor_scalar_mul(nbst[:], nbst[:], -1.0)
            h_sb = hp.tile([128, KO, T], fp8, tag="h")
            for ko in range(KO):
                if b == 0 and ko % 2 == 1:
                    # ramp: ACT is idle pre-attention; halve the h latency.
                    # ACT computes f(in*scale + bias): scale = -rstd,
                    # bias = mean*rstd gives (in - mean)*rstd... sign:
                    # (x - mean)*rstd = x*rstd - mean*rstd.
                    nc.scalar.activation(
                        h_sb[:, ko, :], x_sb[:, ko, :], AF.Identity,
                        scale=bst[:, 2 * ko + 1:2 * ko + 2],
                        bias=nbst[:, ko:ko + 1])
                else:
                    nc.vector.tensor_scalar(
                        out=h_sb[:, ko, :], in0=x_sb[:, ko, :],
                        scalar1=bst[:, 2 * ko:2 * ko + 1], scalar2=bst[:, 2 * ko + 1:2 * ko + 2],
                        op0=ALU.subtract, op1=ALU.mult)
            h_tiles[b] = h_sb
            # only pair 0's q/k upfront; the rest (and v^T) interleave into
            # this batch's attention heads 1-4 to fill exp-gated PE gaps
            for i, m in enumerate((4, 0)):
                emit_qkv_chunk(b, m, on_dve=(b == 0 and i % 2 == 1))

        def emit_qkv_chunk(b, m, on_dve=False):
            h_sb = h_tiles[b]
            q2 = q2s[b]
            for half in range(2):
                sl = slice(512 * half, 512 * (half + 1))
                pq = psQ.tile([128, 512], f32, tag="q")
                for j in range(2):
                    mm(pq[:], wqk_sb[:, 2 * j:2 * j + 2, 128 * m:128 * (m + 1)],
                       h_sb[:, 2 * j:2 * j + 2, sl],
                       start=(j == 0), stop=(j == 1), perf_mode=DR)
                if m < 4:
                    # q' = psum/8 + bq' (Wq host-scaled x64*scale, bias x8*scale)
                    if on_dve or b > 0:
                        nc.vector.tensor_scalar(
                            out=q2[:, 0, m, sl], in0=pq[:],
                            scalar1=0.125, scalar2=bq_sb[:, m:m + 1],
                            op0=ALU.mult, op1=ALU.add)
                    else:
                        nc.scalar.activation(
                            q2[:, 0, m, sl], pq[:], AF.Identity,
                            bias=bq_sb[:, m:m + 1], scale=0.125)
                else:
                    # k' = psum (Wk host-scaled x8); k bias dropped.  One
                    # full-partition psum->sbuf convert, then two byte-move
                    # DMAs scatter the head halves into kz2 (saves half the
                    # ACT/DVE rows vs two [64, 512] copies).
                    p = m - 4
                    kst = ksp.tile([128, 512], fp8, tag="kst")
                    if on_dve:
                        nc.vector.tensor_copy(kst[:], pq[:])
                    else:
                        nc.scalar.copy(kst[:], pq[:])
                    nc.sync.dma_start(kz2[0:64, 0, 2 * p, sl], kst[0:64, :])
                    nc.sync.dma_start(kz2[64:128, 0, 2 * p + 1, sl], kst[64:128, :])

        def emit_vt(b):
            h_sb = h_tiles[b]
            for tc_i in range(8):
                pv = psQ.tile([128, 512], f32, tag="q")
                for j in range(2):
                    mm(pv[:], h_sb[:, 2 * j:2 * j + 2, 128 * tc_i:128 * (tc_i + 1)],
                       wv_sb[:, 2 * j:2 * j + 2, :],
                       start=(j == 0), stop=(j == 1), perf_mode=DR)
                # one strided copy per tc: [p-pair, even/odd, 64] blocks
                pvv = pv[:].rearrange("p (a e c) -> p a e c", a=4, e=2)
                nc.scalar.copy(
                    vt_sb[:, tc_i, :, :].rearrange(
                        "p a (e c) -> p a e c", e=3)[:, :, 0:3:2, :],
                    pvv[:])

        def emit_exp(idx, es_slice, st):
            if _exp_on_act(idx):
                nc.scalar.activation(es_slice, st[:], AF.Exp,
                                     bias=bact_sb[:, 0:1], scale=1.0 / 64.0)
            else:
                nc.vector.tensor_scalar(
                    out=es_slice.bitcast(u8), in0=st[:],
                    scalar1=A_CONST, scalar2=0.0, op0=ALU.mult, op1=ALU.max)

        def emit_attn(b, inter=None):
            """Attention for batch b; returns a_sb.  `inter` is a mutable
            list of emit-closures (prev batch's proj units) drained a couple
            per head to fill the exp-gated PE gaps."""
            if inter is None:
                inter = []
            q2 = q2s[b]

            a_sb = ap_.tile([128, KO, T], fp8, tag="a")
            avs_list = [None] * NH
            rb_pair = [None] * (NH // 2)
            dc_sb = [dcp.tile([4, 512], f32, tag=f"dc{p}", name=f"dc{p}")
                     for p in range(NH // 2)]

            def av_mms(avp, h_av, es_pairs, j, halves=(0, 1)):
                # AV DoubleRow over sc-pair j for head h_av
                p, e = h_av // 2, h_av % 2
                es = es_pairs[j]
                for half in halves:
                    mm(avp[half][:], vt_sb[:, 2 * j:2 * j + 2, p, 64 * e:64 * e + 128],
                       es[:, :, 512 * half:512 * (half + 1)],
                       start=(j == 0), stop=(j == 3), perf_mode=DR)

            def finish_head(h_av, av, split=False):
                e = h_av % 2
                b1 = 64 * (1 - e)
                avs = avsp.tile([128, T], f32, tag="avs")
                for half in range(2):
                    if (split and half == 1) or h_av % 4 == 1:
                        nc.vector.tensor_copy(
                            avs[:, 512 * half:512 * (half + 1)], av[half][:])
                    else:
                        nc.scalar.copy(
                            avs[:, 512 * half:512 * (half + 1)], av[half][:])
                dc = dc_sb[h_av // 2]
                r0 = 2 * (h_av % 2)
                for half in range(2):
                    nc.sync.dma_start(
                        out=dc[r0 + half:r0 + half + 1, :],
                        in_=avs[b1:b1 + 1, 512 * half:512 * (half + 1)])
                avs_list[h_av] = avs

            def emit_chain(p, eng=None, tail=False):
                # reciprocal chain for ONE head pair (4 denominator rows):
                # keeps tail latency to the last pair's chain only
                if eng is None:
                    eng = nc.gpsimd
                dD = dc_sb[p][:, :]
                y = yp.tile([4, 512], f32, tag="y")
                t = yp.tile([4, 512], f32, tag="t")
                eng.tensor_tensor(
                    out=y[:].bitcast(i32),
                    in0=magic_sb[0:4, 0:1].to_broadcast((4, 512)),
                    in1=dD.bitcast(i32), op=ALU.subtract)
                eng.tensor_mul(t[:], dD, y[:])
                nc.vector.scalar_tensor_tensor(
                    out=y[:], in0=t[:], scalar=2.0, in1=y[:],
                    op0=ALU.subtract, op1=ALU.mult)
                if tail:
                    # tail: broadcast 1/D across partitions with a tiny PE
                    # matmul into PSUM (saves 3 serial DMA hops), a-mults on
                    # DVE read the psum directly.
                    # 4 denominator rows share one 512-col block; the rbc
                    # lhsT selects rows by parity per half
                    nc.vector.tensor_scalar_mul(yt_sb[0:4, :], y[:], -1.0)
                    rb_ps = [psB.tile([128, 512], f32, tag=f"av{i}",
                                      name=f"rbps{i}") for i in range(2)]
                    for half in range(2):
                        mm(rb_ps[half][:], rbc_sb[:, half, :], yt_sb[:],
                           start=True, stop=True)
                    for hh in range(2):
                        b0 = 64 * hh
                        for half in range(2):
                            sl = slice(512 * half, 512 * (half + 1))
                            nc.vector.tensor_tensor(
                                out=a_sb[b0:b0 + 64, p, sl],
                                in0=avs_list[2 * p + hh][b0:b0 + 64, sl],
                                in1=rb_ps[half][b0:b0 + 64, :], op=ALU.mult)
                    return
                nc.vector.tensor_scalar_mul(y[:], y[:], -1.0)
                rd = rdp.tile([2, 1024], f32, tag="rd")
                nc.sync.dma_start(out=rd[:], in_=y[:])
                for hh in range(2):
                    h_av = 2 * p + hh
                    e = hh
                    b0 = 64 * e
                    if e == 0:
                        rb_pair[p] = rbp.tile([128, T], f32, tag="rb",
                                              name=f"rb{p}")
                    rB = rb_pair[p]
                    nc.sync.dma_start(
                        out=rB[b0:b0 + 64, :],
                        in_=rd[hh:hh + 1, :].to_broadcast((64, T)))
                    nc.gpsimd.tensor_tensor(
                        out=a_sb[b0:b0 + 64, p, :],
                        in0=avs_list[h_av][b0:b0 + 64, :],
                        in1=rB[b0:b0 + 64, :], op=ALU.mult)

            # Software-pipelined attention as the bf16 kernel: head h's
            # St/exp stream interleaves head h-1's AV pairs.
            prev = None  # (head, es_pairs)
            for h in range(NH):
                last = h == NH - 1
                if h == 1:
                    emit_vt(b)
                if h in (2, 3, 4):
                    for m in ((5, 1), (6, 2), (7, 3))[h - 2]:
                        emit_qkv_chunk(b, m)

                avp = None
                if prev is not None:
                    avp = [psB.tile([128, 512], f32, tag=f"av{i}", name=f"av{i}")
                           for i in range(2)]
                avpL = None
                if last:
                    avpL = [psB.tile([128, 512], f32, tag=f"av{i}",
                                     name=f"av{i}l") for i in range(2)]
                p = h // 2
                es_pairs = []
                for sc in range(8):
                    if sc % 2 == 0:
                        es = esp.tile([128, 2, T], fp8, tag="es")
                        es_pairs.append(es)
                    st = psS.tile([128, T], f32, tag="st")
                    for half in range(2):
                        mm(st[:, 512 * half:512 * (half + 1)],
                           kz2[:, :, h, 128 * sc:128 * (sc + 1)],
                           q2[:, :, p, 512 * half:512 * (half + 1)],
                           start=True, stop=True, perf_mode=DR)
                    emit_exp(h * 8 + sc, es_pairs[sc // 2][:, sc % 2, :], st)
                    if h >= 1 and sc in (2, 5) and inter:
                        inter.pop(0)()
                    if avp is not None:
                        # one AV half-matmul per sc slot: pair sc//2 of the
                        # prev head, half = sc%2 (finer PE fill)
                        av_mms(avp, prev[0], prev[1], sc // 2, halves=(sc % 2,))
                    if last and sc >= 2:
                        av_mms(avpL, h, es_pairs, (sc - 2) // 2,
                               halves=((sc - 2) % 2,))
                if avp is not None:
                    finish_head(prev[0], avp)
                    if prev[0] % 2 == 1:
                        emit_chain(prev[0] // 2)
                        if prev[0] == 3 and b == BPC - 1:
                            # last batch: its own proj j=0 group (a pairs
                            # 0,1 just became available) joins the
                            # interleave queue for heads 5-7
                            inter.extend(proj_units(b, a_sb, (0,), False))
                prev = (h, es_pairs)
            av_mms(avpL, prev[0], prev[1], 3)  # both halves of last pair
            finish_head(prev[0], avpL, split=(b == BPC - 1))
            emit_chain(3, eng=nc.vector if b == BPC - 1 else None,
                       tail=(b == BPC - 1))
            for u in inter:
                u()
            if b == BPC - 1:
                # tail: j=1 + bias group, gated pair (ko 3) first
                for u in proj_units(b, a_sb, (1, "bias"), True,
                                    m_order=(3, 0, 1, 2)):
                    u()
            return a_sb

        def proj_units(b, a_sb, js, lastg, m_order=None):
            # proj via DoubleRow ko-pairs plus a constant bias step: lhsT =
            # bppat (64*bp at partition 0 of slice 1), rhs = q2's ones slice
            # -- adds 64*bp to the psum inside the matmul, so the residual
            # scalar_tensor_tensor (po/64 + x) applies bp for free.
            # Returns a list of emit-closures, one per (m, half) unit, for
            # interleaving into the next batch's attention stream.
            x_sb = x_tiles[b]
            units = []
            for m in (m_order if m_order is not None else range(KO)):
                for half in range(2):
                    def emit(m=m, half=half, js=js, lastg=lastg):
                        sl = slice(512 * half, 512 * (half + 1))
                        po = psQ.tile([128, 512], f32, tag="q")
                        for i, j in enumerate(js):
                            if j == "bias":
                                mm(po[:], bpp_sb[:, :, 128 * m:128 * (m + 1)],
                                   q2s[b][:, :, 0, sl],
                                   start=(i == 0), stop=(i == len(js) - 1),
                                   perf_mode=DR)
                            else:
                                mm(po[:], wp_sb[:, 2 * j:2 * j + 2, 128 * m:128 * (m + 1)],
                                   a_sb[:, 2 * j:2 * j + 2, sl],
                                   start=(i == 0), stop=(i == len(js) - 1),
                                   perf_mode=DR)
                        nc.vector.scalar_tensor_tensor(
                            out=x_sb[:, m, sl], in0=po[:], scalar=1.0 / 64.0,
                            in1=x_sb[:, m, sl], op0=ALU.mult, op1=ALU.add)
                        if lastg and half == 1:
                            nc.sync.dma_start(out_d[b, :, m, :], x_sb[:, m, :])
                    units.append(emit)
            return units

        emit_gn_qkv(0)
        a_tiles = [None] * BPC
        for b in range(BPC):
            if b + 1 < BPC:
                x_tiles[b + 1] = emit_x_load(b + 1)
            inter = []
            if b > 0:
                inter = proj_units(b - 1, a_tiles[b - 1], (0, 1, "bias"), True)
            a_tiles[b] = emit_attn(b, inter)
            if DEBUG_LIGHT:
                nc.sync.dma_start(dbg_h[b], h_tiles[b][:])
                nc.sync.dma_start(dbg_a[b], a_tiles[b][:])
            if b + 1 < BPC:
                emit_gn_qkv(b + 1)

    if not nc.is_finalized():
        nc.finalize()
    return nc


def _prep_inputs(x, norm_w, norm_b, qkv_w, qkv_b, proj_w, proj_b):
    """Fold norms/biases/scales into weights; reshape for the kernel layout."""
    f = np.float32
    x = np.asarray(x, f)
    nw = np.asarray(norm_w, f)
    nb = np.asarray(norm_b, f)
    qkv_w = np.asarray(qkv_w, f)
    qkv_b = np.asarray(qkv_b, f)
    proj_w = np.asarray(proj_w, f)
    proj_b = np.asarray(proj_b, f)

    Wq, Wk, Wv = qkv_w[0:C], qkv_w[C:2 * C], qkv_w[2 * C:3 * C]
    bqv, bkv, bvv = qkv_b[0:C], qkv_b[C:2 * C], qkv_b[2 * C:3 * C]
    scale = f(1.0 / np.sqrt(CH))
    # fp8 scaling: q' = 8q, k' = 8k, v' = 8v; Wq gets an extra x8 (undone in
    # the q psum copy) so its fp8 entries sit in the normal range.
    Wq_e = (Wq * nw[None, :]) * scale * 64.0
    bq_e = (Wq @ nb + bqv) * scale * 8.0
    Wk_e = (Wk * nw[None, :]) * 8.0   # k bias dropped (softmax shift invariance)
    Wv_e = (Wv * nw[None, :]) * 8.0
    bv_e = Wv @ nb + bvv
    bp_e = proj_b + proj_w @ bv_e     # v bias folded into proj bias
    Wp_e = proj_w * 8.0

    def chan_chunks(vec):  # [C] -> [128, KO]
        return np.ascontiguousarray(vec.reshape(KO, 128).T)

    def lhsT_chunks(wT, dtype):  # [C, M] -> [128, KO, M]
        return np.ascontiguousarray(
            wT.reshape(KO, 128, wT.shape[1]).transpose(1, 0, 2)).astype(dtype)

    wqkT = np.concatenate([Wq_e, Wk_e], axis=0).T  # [C, 1024]
    gm = np.zeros((C, NG), f)
    gm[np.arange(C), np.arange(C) // (C // NG)] = 1.0 / (C // NG)
    bm = np.zeros((128, C), f)
    bm[np.arange(C) // (C // NG), np.arange(C)] = 1.0

    d8 = _np8()
    bpp = np.zeros((128, 2, C), np.float32)
    bpp[0, 1, :] = 64.0 * bp_e
    pat = np.zeros((128, 3 * T), d8)
    pat[0, 0:T] = d8.type(BETA)
    pat[0, T:2 * T] = d8.type(1.0)
    rbc = np.zeros((128, 2, 128), np.float32)
    for m in range(128):
        for half in range(2):
            rbc[2 * (m // 64) + half, half, m] = 1.0
    shared = {
        "pat": pat,
        "rbc": rbc.astype(_npbf()),
        "wqkT": lhsT_chunks(wqkT, d8),
        "wvT": lhsT_chunks(Wv_e.T, d8),
        "wpT": lhsT_chunks(Wp_e.T, d8),
        "bq": chan_chunks(bq_e),
        "bppat": bpp.astype(d8),
        "gmat": np.ascontiguousarray(
            gm.reshape(KO, 128, NG).transpose(1, 0, 2)),
        "bmat": np.ascontiguousarray(bm.reshape(128, KO, 128)),
        "ones": np.ones((128, 64), d8),
    }
    xr = x.reshape(B, C, T)
    in_maps = []
    for c in range(NCORES):
        xc = xr[c * BPC:(c + 1) * BPC].reshape(BPC, KO, 128, T).transpose(0, 2, 1, 3)
        m = dict(shared)
        m["x"] = np.ascontiguousarray(xc).astype(_npbf())
        in_maps.append(m)
    return in_maps


def kernel(x, norm_w, norm_b, qkv_w, qkv_b, proj_w, proj_b):
    from concourse.bass_utils import run_bass_kernel_spmd

    in_maps = _prep_inputs(x, norm_w, norm_b, qkv_w, qkv_b, proj_w, proj_b)
    nc = _build_nc()
    res = run_bass_kernel_spmd(nc, in_maps, core_ids=list(range(NCORES)), trace=TRACE)
    kernel.last_results = res
    outs = []
    for c in range(NCORES):
        oc = res.results[c]["out"]  # [BPC, 128, KO, T]
        outs.append(np.asarray(oc).transpose(0, 2, 1, 3).reshape(BPC, C, T))
    full = np.concatenate(outs, axis=0).reshape(B, C, 32, 32).astype(np.float32)
    return full


# revision 30
# speedup vs baseline: 1.0212x; 1.0212x over previous
"""AttentionBlock kernel for 8 Trainium2 NeuronCores (Bass/Tile), fp8 edition.

Problem (hardcoded shapes): x [16, 512, 32, 32] fp32, GroupNorm(32 groups,
eps=1e-5) -> 1x1-conv QKV (qkv_w [1536,512], qkv_b) -> 8-head attention over
T=1024 positions (head dim 64) -> 1x1-conv proj -> residual add.

Sharding: pure data-parallel over batch; each of the 8 cores handles 2
batches end-to-end; weights replicated; no collectives.

All five matmul families (QKV, v^T, St, AV, proj) run in fp8e4m3 with
MatmulPerfMode.DoubleRow: both operands packed [K, 2, .], one instruction
contracts 256 rows at the PE's fixed 1 output row/cycle (2x bf16 work per
instruction for the contraction-bound families; St, which is output-bound,
costs the same as bf16 but the spare DR slice carries a constant).

Scaling scheme (fp8 e4m3 here: bias 7, max 240, bits 120+ are inf):
  q' = 8q, k' = 8k, v' = 8v carried in fp8 (weights pre-scaled x8 on host;
  Wq an extra x8 so its fp8 entries stay normal-range, undone in the q
  psum->sbuf copy).  St psum = q'.k' = 64*logit + BETA (see below).  The
  proj matmul computes 64*proj(a) + 64*bp -- the proj bias rides a third
  DoubleRow step whose lhsT is a constant pattern and whose rhs is the q2
  ones slice -- all undone by the 1/64 in the residual
  scalar_tensor_tensor.

St DoubleRow slices: slice0 = zero-padded k' / q' data; slice1 = a rank-1
constant (BETA=160 at partition 0 of kz2, 1.0 at partition 0 of q2) adding
BETA to every logit inside the matmul.  That makes exp-as-fp8-bits a single
op: fp8 bits of exp((P-BETA)/64 - S) equal P*0.18034 for this BETA, so one
DVE tensor_scalar (mult, max-with-0) -> uint8 convert writes the attention
weights directly as fp8 bits (Schraudolph).  The other share of tiles uses
the real ACT exp table (scale=1/64, bias=-S-BETA/64); the two es families
agree up to softmax-invariant constants.  sc-pairs split engines so each
AV DoubleRow rhs pair is ready in one exp latency.  GPSIMD cannot touch
PSUM on this HW (and its TensorScalarPtr path is ~17x slow), so the
softmax + all psum->sbuf drains live on ACT+DVE only; GPSIMD gets
SBUF-only work (a-mults, reciprocal seeds, h-norm stays DVE).

Denominators come free (ones columns in the AV lhsT, replicated on the
dead 64 partitions), one magic-seed+Newton reciprocal chain per HEAD PAIR
(so only the last pair's chain sits on the tail), broadcast via a DRAM
bounce mid-run; the tail pair instead broadcasts 1/D across partitions
with a tiny PE matmul into PSUM (saves three serial DMA hops) and its
a-mults run on DVE.  proj(b-1) is cut into per-(m,half) units drained two
per head inside batch b's attention to fill the exp-gated PE gaps; the
last batch's proj j=0 group joins that queue after pair 1's chain.
GroupNorm runs in f32 (bn_stats + tiny constant matmuls, rsqrt via Newton
so ACT keeps only the exp table loaded).  Large constant patterns (kz2
pads/beta, q2 ones) are DMA-broadcast from a small host tensor instead of
memset (engine memsets are expensive; inputs stay slim for the load
barrier).
"""

import numpy as np

B, C, T = 16, 512, 1024
NH, CH = 8, 64
NG = 32
EPS = 1e-5
NCORES = 8
BPC = B // NCORES  # batches per core
KO = C // 128      # channel chunks

TRACE = False
DEBUG_LIGHT = False  # only h + a outputs (minimal schedule perturbation)

# fp8 softmax constants (see module docstring)
BETA = 160.0
A_CONST = float(8 * np.log2(np.e) / 64.0)          # 0.18033688
S_EFF = float((56.5 - 8 * np.log2(np.e) * (BETA / 64.0)) / (8 * np.log2(np.e)))
B_ACT = float(-S_EFF - BETA / 64.0)

# exp engine routing (A=ACT exp table, D=DVE Schraudolph); GPSIMD cannot
# access PSUM on this HW, so softmax drains only through ACT/DVE.  N_ACT_EXP
# of the 128 tiles (2 batches x 64) go to ACT, evenly spread.
N_EXP = 2 * NH * 8
N_ACT_EXP = 72


def _exp_on_act(idx):
    # each sc-pair's two exps go to different engines so the pair is ready
    # in one exp-latency; extra ACT share on alternate heads (~56% ACT)
    h, sc = (idx // 8) % NH, idx % 8
    if sc == 7 and h % 2 == 0:
        return True
    return sc % 2 == 0


def _np8():
    import ml_dtypes
    return np.dtype(ml_dtypes.float8_e4m3)


def _npbf():
    import ml_dtypes
    return np.dtype(ml_dtypes.bfloat16)


def _build_nc():
    import concourse.bass as bass
    import concourse.tile as tile
    from concourse import bacc, mybir
    from contextlib import ExitStack

    f32 = mybir.dt.float32
    bf16 = mybir.dt.bfloat16
    fp8 = mybir.dt.float8e4
    u8 = mybir.dt.uint8
    i32 = mybir.dt.int32
    DR = mybir.MatmulPerfMode.DoubleRow

    nc = bacc.Bacc()
    AF = mybir.ActivationFunctionType
    ALU = mybir.AluOpType

    x_d = nc.dram_tensor("x", [BPC, 128, KO, T], bf16, kind="ExternalInput")
    wqk_d = nc.dram_tensor("wqkT", [128, KO, 2 * C], fp8, kind="ExternalInput")
    wv_d = nc.dram_tensor("wvT", [128, KO, C], fp8, kind="ExternalInput")
    wp_d = nc.dram_tensor("wpT", [128, KO, C], fp8, kind="ExternalInput")
    bq_d = nc.dram_tensor("bq", [128, KO], f32, kind="ExternalInput")
    bpp_d = nc.dram_tensor("bppat", [128, 2, C], fp8, kind="ExternalInput")
    g_d = nc.dram_tensor("gmat", [128, KO, NG], f32, kind="ExternalInput")
    b_d = nc.dram_tensor("bmat", [128, KO, 128], f32, kind="ExternalInput")
    ones_d = nc.dram_tensor("ones", [128, 64], fp8, kind="ExternalInput")
    rbc_d = nc.dram_tensor("rbc", [128, 2, 128], bf16, kind="ExternalInput")
    # pat: cols 0:T beta-at-partition0, T:2T ones-at-partition0, 2T:3T zeros
    pat_d = nc.dram_tensor("pat", [128, 3 * T], fp8, kind="ExternalInput")
    out_d = nc.dram_tensor("out", [BPC, 128, KO, T], bf16, kind="ExternalOutput")
    if DEBUG_LIGHT:
        dbg_h = nc.dram_tensor("dbg_h", [BPC, 128, KO, T], fp8, kind="ExternalOutput")
        dbg_a = nc.dram_tensor("dbg_a", [BPC, 128, KO, T], fp8, kind="ExternalOutput")

    # Every matmul keeps the PE in the default 128-row tiling mode.
    def mm(out, lhsT, rhs, **kw):
        assert lhsT.partition_size() == 128
        return nc.tensor.matmul(out, lhsT, rhs, **kw)

    with tile.TileContext(nc) as tc, ExitStack() as ctx:
        consts = ctx.enter_context(tc.tile_pool(name="consts", bufs=1))
        xp = ctx.enter_context(tc.tile_pool(name="xp", bufs=2))
        hp = ctx.enter_context(tc.tile_pool(name="hp", bufs=2))
        qp = ctx.enter_context(tc.tile_pool(name="qp", bufs=2))
        kzp = ctx.enter_context(tc.tile_pool(name="kzp", bufs=1))
        ksp = ctx.enter_context(tc.tile_pool(name="ksp", bufs=2))
        vtp = ctx.enter_context(tc.tile_pool(name="vtp", bufs=1))
        esp = ctx.enter_context(tc.tile_pool(name="esp", bufs=12))
        avsp = ctx.enter_context(tc.tile_pool(name="avsp", bufs=6))
        rbp = ctx.enter_context(tc.tile_pool(name="rbp", bufs=2))
        dcp = ctx.enter_context(tc.tile_pool(name="dcp", bufs=2))
        yp = ctx.enter_context(tc.tile_pool(name="yp", bufs=2))
        ap_ = ctx.enter_context(tc.tile_pool(name="ap", bufs=2))
        gnp = ctx.enter_context(tc.tile_pool(name="gnp", bufs=2))
        psS = ctx.enter_context(tc.tile_pool(name="psS", bufs=2, space="PSUM"))
        psB = ctx.enter_context(tc.tile_pool(name="psB", bufs=1, space="PSUM"))
        psQ = ctx.enter_context(tc.tile_pool(name="psQ", bufs=2, space="PSUM"))
        rdp = ctx.enter_context(tc.tile_pool(name="rdp", bufs=4, space="DRAM"))

        # ---- small constants first (GroupNorm needs only these + x) ----
        bq_sb = consts.tile([128, KO], f32)
        nc.sync.dma_start(bq_sb[:], bq_d[:])
        bpp_sb = consts.tile([128, 2, C], fp8)
        nc.sync.dma_start(bpp_sb[:], bpp_d[:])
        g_sb = consts.tile([128, KO, NG], f32)
        nc.sync.dma_start(g_sb[:], g_d[:])
        bm_sb = consts.tile([128, KO, 128], f32)
        # ACT exp scale/bias scalars
        bact_sb = consts.tile([128, 2], f32)
        nc.gpsimd.memset(bact_sb[:, 0:1], B_ACT)
        nc.gpsimd.memset(bact_sb[:, 1:2], 1.0 / 64.0)
        # Magic seed constant for the Newton reciprocal (fast-inverse trick).
        magic_sb = consts.tile([128, 2], i32)
        nc.gpsimd.memset(magic_sb[:], 0x7EF127EA)
        # Magic seed for Newton rsqrt.
        magic_rs = consts.tile([NG, 1], i32)
        nc.gpsimd.memset(magic_rs[:], 0x5F3759DF)
        # tail reciprocal-broadcast: lhsT patterns + a 128-partition y tile
        # (rows 4:128 zeroed once; matmul rhs must be garbage-free)
        rbc_sb = consts.tile([128, 2, 128], bf16)
        nc.sync.dma_start(rbc_sb[:], rbc_d[:])
        yt_sb = consts.tile([128, 512], bf16)
        nc.gpsimd.memset(yt_sb[:], 0.0)

        # ---- batch 0 input before the big weights ----
        def emit_x_load(b):
            x_sb = xp.tile([128, KO, T], bf16, tag="x")
            for ko in range(KO):
                nc.sync.dma_start(x_sb[:, ko, :], x_d[b, :, ko, :])
            return x_sb

        x_tiles = [None] * BPC
        x_tiles[0] = emit_x_load(0)
        nc.sync.dma_start(bm_sb[:], b_d[:])

        wqk_sb = consts.tile([128, KO, 2 * C], fp8)
        nc.sync.dma_start(wqk_sb[:], wqk_d[:])
        wv_sb = consts.tile([128, KO, C], fp8)
        nc.sync.dma_start(wv_sb[:], wv_d[:])
        wp_sb = consts.tile([128, KO, C], fp8)
        nc.sync.dma_start(wp_sb[:], wp_d[:])

        # kz2 [128, 2, NH, T]: slice0 = zero-padded k' per head (head h's k'
        # on partitions 64*(h%2)..+64, zeros elsewhere); slice1 = beta
        # pattern (beta at partition 0, zeros elsewhere).  DMA-initialized
        # (pads on one queue, beta slice on another).
        kz2 = kzp.tile([128, 2, NH, T], fp8, tag="kz")
        # slice1 = beta pattern broadcast over heads; slice0 pads = zeros
        nc.sync.dma_start(
            kz2[:, 1, :, :],
            bass.AP(tensor=pat_d, offset=0, ap=[[3 * T, 128], [0, NH], [1, T]]))
        nc.sync.dma_start(
            kz2[64:128, 0, 0:NH:2, :],
            bass.AP(tensor=pat_d, offset=64 * 3 * T + 2 * T,
                    ap=[[3 * T, 64], [0, NH // 2], [1, T]]))
        nc.sync.dma_start(
            kz2[0:64, 0, 1:NH:2, :],
            bass.AP(tensor=pat_d, offset=2 * T,
                    ap=[[3 * T, 64], [0, NH // 2], [1, T]]))

        # q2 per batch [128, 2, KO, T]: slice0 = q' data, slice1 = ones
        # pattern (1.0 at partition 0).
        q2s = []
        for b in range(BPC):
            q2 = qp.tile([128, 2, KO, T], fp8, tag="q", name=f"q{b}")
            nc.sync.dma_start(
                q2[:, 1, :, :],
                bass.AP(tensor=pat_d, offset=T,
                        ap=[[3 * T, 128], [0, KO], [1, T]]))
            q2s.append(q2)
        # v^T lhsT buffer: per head-pair p the 192 columns are
        # [vT_even(64) | ones(64) | vT_odd(64)]; head 2p uses cols 0:128 and
        # head 2p+1 uses cols 64:192.  Ones blocks DMA'd once.
        vt_sb = vtp.tile([128, 8, 4, 192], fp8, tag="vt")
        ones_src = bass.AP(tensor=ones_d, offset=0,
                           ap=[[64, 128], [0, 32], [1, 64]])
        vt_flat = vt_sb[:].rearrange("p a b w -> p (a b) w")
        nc.sync.dma_start(vt_flat[:, :, 64:128], ones_src)

        # PE p-state warmup: harmless matmuls on already-loaded constants
        # during the GroupNorm ramp (PE would otherwise idle and downclock,
        # making the first QKV/St matmuls run at 1.2GHz)
        warm_ps = psQ.tile([128, 512], f32, tag="q")
        for w in range(24):
            mm(warm_ps[:], wqk_sb[:, 0, 0:128], wqk_sb[:, 0, 0:512],
               start=(w == 0), stop=(w == 23))

        # per-batch live tiles
        h_tiles = [None] * BPC

        def emit_gn_qkv(b):
            """GroupNorm + QKV (q,k) + v^T for batch b."""
            x_sb = x_tiles[b]

            # ---------------- GroupNorm (f32, as bf16 kernel) ------------
            rhs3 = gnp.tile([128, KO, 3], f32, tag="rhs3")
            for ko in range(KO):
                stats = gnp.tile([128, 2, 6], f32, tag="stats")
                for j in range(2):
                    nc.vector.bn_stats(out=stats[:, j, :], in_=x_sb[:, ko, 512 * j:512 * (j + 1)])
                nc.vector.bn_aggr(out=rhs3[:, ko, 0:2], in_=stats[:])
                nc.vector.tensor_mul(rhs3[:, ko, 2:3], rhs3[:, ko, 0:1], rhs3[:, ko, 0:1])
            gps = psQ.tile([NG, 3], f32, tag="q")
            for ko in range(KO):
                mm(gps[:], g_sb[:, ko, :], rhs3[:, ko, :],
                   start=(ko == 0), stop=(ko == KO - 1))
            gq = gnp.tile([NG, 3], f32, tag="gq")
            nc.vector.tensor_copy(gq[:], gps[:])
            gtmp = gnp.tile([NG, 2], f32, tag="gtmp")
            gst2 = gnp.tile([128, 2], f32, tag="gst2")
            nc.vector.memset(gst2[:], 0.0)
            nc.vector.tensor_copy(gst2[0:NG, 0:1], gq[:, 0:1])
            nc.vector.tensor_add(gtmp[:, 0:1], gq[:, 1:2], gq[:, 2:3])
            nc.vector.tensor_mul(gtmp[:, 1:2], gq[:, 0:1], gq[:, 0:1])
            nc.vector.tensor_sub(gtmp[:, 0:1], gtmp[:, 0:1], gtmp[:, 1:2])
            vpe = gtmp[:, 0:1]
            nc.vector.tensor_scalar_add(vpe, vpe, EPS)
            rs = gnp.tile([NG, 3], f32, tag="rs")
            ry = rs[:, 0:1]
            ra = rs[:, 1:2]
            rb = rs[:, 2:3]
            nc.vector.tensor_scalar(
                out=ra.bitcast(i32), in0=vpe.bitcast(i32),
                scalar1=1, scalar2=None, op0=ALU.arith_shift_right)
            nc.vector.tensor_tensor(
                out=ry.bitcast(i32), in0=magic_rs[:],
                in1=ra.bitcast(i32), op=ALU.subtract)
            for _ in range(1):
                nc.vector.tensor_mul(ra, vpe, ry)
                nc.vector.tensor_mul(rb, ra, ry)
                nc.vector.scalar_tensor_tensor(
                    out=ra, in0=rb, scalar=3.0, in1=ry,
                    op0=ALU.subtract, op1=ALU.mult)
                nc.vector.tensor_scalar_mul(ry, ra, -0.5)
            nc.vector.tensor_copy(gst2[0:NG, 1:2], ry)
            bst_ps = psQ.tile([128, 2 * KO], f32, tag="q")
            for ko in range(KO):
                mm(bst_ps[:, 2 * ko:2 * ko + 2], bm_sb[:, ko, :], gst2[:],
                   start=True, stop=True)
            bst = gnp.tile([128, 2 * KO], f32, tag="bst_sb")
            nc.vector.tensor_copy(bst[:], bst_ps[:])
            nbst = gnp.tile([128, KO], f32, tag="nbst")
            if b == 0:
                # -mean*rstd for the ACT-side h-norm (bias term)
                nc.vector.tensor_tensor(
                    out=nbst[:], in0=bst[:, 0:2 * KO:2], in1=bst[:, 1:2 * KO:2],
                    op=ALU.mult)
                nc.vector.tensor_scalar_mul(nbst[:], nbst[:], -1.0)
            h_sb = hp.tile([128, KO, T], fp8, tag="h")
            for ko in range(KO):
                if b == 0 and ko % 2 == 1:
                    # ramp: ACT is idle pre-attention; halve the h latency.
                    # ACT computes f(in*scale + bias): scale = -rstd,
                    # bias = mean*rstd gives (in - mean)*rstd... sign:
                    # (x - mean)*rstd = x*rstd - mean*rstd.
                    nc.scalar.activation(
                        h_sb[:, ko, :], x_sb[:, ko, :], AF.Identity,
                        scale=bst[:, 2 * ko + 1:2 * ko + 2],
                        bias=nbst[:, ko:ko + 1])
                else:
                    nc.vector.tensor_scalar(
                        out=h_sb[:, ko, :], in0=x_sb[:, ko, :],
                        scalar1=bst[:, 2 * ko:2 * ko + 1], scalar2=bst[:, 2 * ko + 1:2 * ko + 2],
                        op0=ALU.subtract, op1=ALU.mult)
            h_tiles[b] = h_sb
            order = (4, 0) if b == 0 else (4, 0, 5, 1, 6, 2, 7, 3)
            for i, m in enumerate(order):
                emit_qkv_chunk(b, m, on_dve=(b == 0 and i % 2 == 1))
            if b > 0:
                emit_vt(b)

        def emit_qkv_chunk(b, m, on_dve=False):
            h_sb = h_tiles[b]
            q2 = q2s[b]
            for half in range(2):
                sl = slice(512 * half, 512 * (half + 1))
                pq = psQ.tile([128, 512], f32, tag="q")
                for j in range(2):
                    mm(pq[:], wqk_sb[:, 2 * j:2 * j + 2, 128 * m:128 * (m + 1)],
                       h_sb[:, 2 * j:2 * j + 2, sl],
                       start=(j == 0), stop=(j == 1), perf_mode=DR)
                if m < 4:
                    # q' = psum/8 + bq' (Wq host-scaled x64*scale, bias x8*scale)
                    if on_dve or b > 0:
                        nc.vector.tensor_scalar(
                            out=q2[:, 0, m, sl], in0=pq[:],
                            scalar1=0.125, scalar2=bq_sb[:, m:m + 1],
                            op0=ALU.mult, op1=ALU.add)
                    else:
                        nc.scalar.activation(
                            q2[:, 0, m, sl], pq[:], AF.Identity,
                            bias=bq_sb[:, m:m + 1], scale=0.125)
                else:
                    # k' = psum (Wk host-scaled x8); k bias dropped.  One
                    # full-partition psum->sbuf convert, then two byte-move
                    # DMAs scatter the head halves into kz2 (saves half the
                    # ACT/DVE rows vs two [64, 512] copies).
                    p = m - 4
                    kst = ksp.tile([128, 512], fp8, tag="kst")
                    if on_dve:
                        nc.vector.tensor_copy(kst[:], pq[:])
                    else:
                        nc.scalar.copy(kst[:], pq[:])
                    nc.sync.dma_start(kz2[0:64, 0, 2 * p, sl], kst[0:64, :])
                    nc.sync.dma_start(kz2[64:128, 0, 2 * p + 1, sl], kst[64:128, :])

        def emit_vt(b):
            h_sb = h_tiles[b]
            for tc_i in range(8):
                pv = psQ.tile([128, 512], f32, tag="q")
                for j in range(2):
                    mm(pv[:], h_sb[:, 2 * j:2 * j + 2, 128 * tc_i:128 * (tc_i + 1)],
                       wv_sb[:, 2 * j:2 * j + 2, :],
                       start=(j == 0), stop=(j == 1), perf_mode=DR)
                # one strided copy per tc: [p-pair, even/odd, 64] blocks
                pvv = pv[:].rearrange("p (a e c) -> p a e c", a=4, e=2)
                nc.scalar.copy(
                    vt_sb[:, tc_i, :, :].rearrange(
                        "p a (e c) -> p a e c", e=3)[:, :, 0:3:2, :],
                    pvv[:])

        def emit_exp(idx, es_slice, st):
            if _exp_on_act(idx):
                nc.scalar.activation(es_slice, st[:], AF.Exp,
                                     bias=bact_sb[:, 0:1], scale=1.0 / 64.0)
            else:
                nc.vector.tensor_scalar(
                    out=es_slice.bitcast(u8), in0=st[:],
                    scalar1=A_CONST, scalar2=0.0, op0=ALU.mult, op1=ALU.max)

        def emit_attn(b, inter=None):
            """Attention for batch b; returns a_sb.  `inter` is a mutable
            list of emit-closures (prev batch's proj units) drained a couple
            per head to fill the exp-gated PE gaps."""
            if inter is None:
                inter = []
            q2 = q2s[b]

            a_sb = ap_.tile([128, KO, T], fp8, tag="a")
            avs_list = [None] * NH
            rb_pair = [None] * (NH // 2)
            dc_sb = [dcp.tile([4, 512], f32, tag=f"dc{p}", name=f"dc{p}")
                     for p in range(NH // 2)]

            def av_mms(avp, h_av, es_pairs, j, halves=(0, 1)):
                # AV DoubleRow over sc-pair j for head h_av
                p, e = h_av // 2, h_av % 2
                es = es_pairs[j]
                for half in halves:
                    mm(avp[half][:], vt_sb[:, 2 * j:2 * j + 2, p, 64 * e:64 * e + 128],
                       es[:, :, 512 * half:512 * (half + 1)],
                       start=(j == 0), stop=(j == 3), perf_mode=DR)

            def finish_head(h_av, av, split=False):
                e = h_av % 2
                b1 = 64 * (1 - e)
                avs = avsp.tile([128, T], f32, tag="avs")
                for half in range(2):
                    if (split and half == 1) or h_av % 4 == 1:
                        nc.vector.tensor_copy(
                            avs[:, 512 * half:512 * (half + 1)], av[half][:])
                    else:
                        nc.scalar.copy(
                            avs[:, 512 * half:512 * (half + 1)], av[half][:])
                dc = dc_sb[h_av // 2]
                r0 = 2 * (h_av % 2)
                for half in range(2):
                    nc.sync.dma_start(
                        out=dc[r0 + half:r0 + half + 1, :],
                        in_=avs[b1:b1 + 1, 512 * half:512 * (half + 1)])
                avs_list[h_av] = avs

            def emit_chain(p, eng=None, tail=False):
                # reciprocal chain for ONE head pair (4 denominator rows):
                # keeps tail latency to the last pair's chain only
                if eng is None:
                    eng = nc.gpsimd
                dD = dc_sb[p][:, :]
                y = yp.tile([4, 512], f32, tag="y")
                t = yp.tile([4, 512], f32, tag="t")
                eng.tensor_tensor(
                    out=y[:].bitcast(i32),
                    in0=magic_sb[0:4, 0:1].to_broadcast((4, 512)),
                    in1=dD.bitcast(i32), op=ALU.subtract)
                eng.tensor_mul(t[:], dD, y[:])
                nc.vector.scalar_tensor_tensor(
                    out=y[:], in0=t[:], scalar=2.0, in1=y[:],
                    op0=ALU.subtract, op1=ALU.mult)
                if tail:
                    # tail: broadcast 1/D across partitions with a tiny PE
                    # matmul into PSUM (saves 3 serial DMA hops), a-mults on
                    # DVE read the psum directly.
                    # 4 denominator rows share one 512-col block; the rbc
                    # lhsT selects rows by parity per half
                    nc.vector.tensor_scalar_mul(yt_sb[0:4, :], y[:], -1.0)
                    rb_ps = [psB.tile([128, 512], f32, tag=f"av{i}",
                                      name=f"rbps{i}") for i in range(2)]
                    for half in range(2):
                        mm(rb_ps[half][:], rbc_sb[:, half, :], yt_sb[:],
                           start=True, stop=True)
                    for hh in range(2):
                        b0 = 64 * hh
                        for half in range(2):
                            sl = slice(512 * half, 512 * (half + 1))
                            nc.vector.tensor_tensor(
                                out=a_sb[b0:b0 + 64, p, sl],
                                in0=avs_list[2 * p + hh][b0:b0 + 64, sl],
                                in1=rb_ps[half][b0:b0 + 64, :], op=ALU.mult)
                    return
                nc.vector.tensor_scalar_mul(y[:], y[:], -1.0)
                rd = rdp.tile([2, 1024], f32, tag="rd")
                nc.sync.dma_start(out=rd[:], in_=y[:])
                for hh in range(2):
                    h_av = 2 * p + hh
                    e = hh
                    b0 = 64 * e
                    if e == 0:
                        rb_pair[p] = rbp.tile([128, T], f32, tag="rb",
                                              name=f"rb{p}")
                    rB = rb_pair[p]
                    nc.sync.dma_start(
                        out=rB[b0:b0 + 64, :],
                        in_=rd[hh:hh + 1, :].to_broadcast((64, T)))
                    nc.gpsimd.tensor_tensor(
                        out=a_sb[b0:b0 + 64, p, :],
                        in0=avs_list[h_av][b0:b0 + 64, :],
                        in1=rB[b0:b0 + 64, :], op=ALU.mult)

            # Software-pipelined attention as the bf16 kernel: head h's
            # St/exp stream interleaves head h-1's AV pairs.
            prev = None  # (head, es_pairs)
            for h in range(NH):
                last = h == NH - 1
                if h == 1 and b == 0:
                    emit_vt(b)
                if b == 0 and h in (2, 3, 4):
                    for m in ((5, 1), (6, 2), (7, 3))[h - 2]:
                        emit_qkv_chunk(b, m)

                avp = None
                if prev is not None:
                    avp = [psB.tile([128, 512], f32, tag=f"av{i}", name=f"av{i}")
                           for i in range(2)]
                avpL = None
                if last:
                    avpL = [psB.tile([128, 512], f32, tag=f"av{i}",
                                     name=f"av{i}l") for i in range(2)]
                p = h // 2
                es_pairs = []
                for sc in range(8):
                    if sc % 2 == 0:
                        es = esp.tile([128, 2, T], fp8, tag="es")
                        es_pairs.append(es)
                    st = psS.tile([128, T], f32, tag="st")
                    for half in range(2):
                        mm(st[:, 512 * half:512 * (half + 1)],
                           kz2[:, :, h, 128 * sc:128 * (sc + 1)],
                           q2[:, :, p, 512 * half:512 * (half + 1)],
                           start=True, stop=True, perf_mode=DR)
                    emit_exp(h * 8 + sc, es_pairs[sc // 2][:, sc % 2, :], st)
                    if h >= 1 and sc in (2, 5) and inter:
                        inter.pop(0)()
                    if avp is not None:
                        # one AV half-matmul per sc slot: pair sc//2 of the
                        # prev head, half = sc%2 (finer PE fill)
                        av_mms(avp, prev[0], prev[1], sc // 2, halves=(sc % 2,))
                    if last and sc >= 2:
                        av_mms(avpL, h, es_pairs, (sc - 2) // 2,
                               halves=((sc - 2) % 2,))
                if avp is not None:
                    finish_head(prev[0], avp)
                    if prev[0] % 2 == 1:
                        emit_chain(prev[0] // 2)
                        if prev[0] == 3 and b == BPC - 1:
                            # last batch: its own proj j=0 group (a pairs
                            # 0,1 just became available) joins the
                            # interleave queue for heads 5-7
                            inter.extend(proj_units(b, a_sb, (0,), False))
                prev = (h, es_pairs)
            av_mms(avpL, prev[0], prev[1], 3)  # both halves of last pair
            finish_head(prev[0], avpL, split=(b == BPC - 1))
            emit_chain(3, eng=nc.vector if b == BPC - 1 else None,
                       tail=(b == BPC - 1))
            for u in inter:
                u()
            if b == BPC - 1:
                # tail: j=1 + bias group, gated pair (ko 3) first
                for u in proj_units(b, a_sb, (1, "bias"), True,
                                    m_order=(3, 0, 1, 2)):
                    u()
            return a_sb

        def proj_units(b, a_sb, js, lastg, m_order=None):
            # proj via DoubleRow ko-pairs plus a constant bias step: lhsT =
            # bppat (64*bp at partition 0 of slice 1), rhs = q2's ones slice
            # -- adds 64*bp to the psum inside the matmul, so the residual
            # scalar_tensor_tensor (po/64 + x) applies bp for free.
            # Returns a list of emit-closures, one per (m, half) unit, for
            # interleaving into the next batch's attention stream.
            x_sb = x_tiles[b]
            units = []
            for m in (m_order if m_order is not None else range(KO)):
                for half in range(2):
                    def emit(m=m, half=half, js=js, lastg=lastg):
                        sl = slice(512 * half, 512 * (half + 1))
                        po = psQ.tile([128, 512], f32, tag="q")
                        for i, j in enumerate(js):
                            if j == "bias":
                                mm(po[:], bpp_sb[:, :, 128 * m:128 * (m + 1)],
                                   q2s[b][:, :, 0, sl],
                                   start=(i == 0), stop=(i == len(js) - 1),
                                   perf_mode=DR)
                            else:
                                mm(po[:], wp_sb[:, 2 * j:2 * j + 2, 128 * m:128 * (m + 1)],
                                   a_sb[:, 2 * j:2 * j + 2, sl],
                                   start=(i == 0), stop=(i == len(js) - 1),
                                   perf_mode=DR)
                        nc.vector.scalar_tensor_tensor(
                            out=x_sb[:, m, sl], in0=po[:], scalar=1.0 / 64.0,
                            in1=x_sb[:, m, sl], op0=ALU.mult, op1=ALU.add)
                        if lastg and half == 1:
                            nc.sync.dma_start(out_d[b, :, m, :], x_sb[:, m, :])
                    units.append(emit)
            return units

        emit_gn_qkv(0)
        a_tiles = [None] * BPC
        for b in range(BPC):
            if b + 1 < BPC:
                x_tiles[b + 1] = emit_x_load(b + 1)
            inter = []
            if b > 0:
                inter = proj_units(b - 1, a_tiles[b - 1], (0, 1, "bias"), True)
            a_tiles[b] = emit_attn(b, inter)
            if DEBUG_LIGHT:
                nc.sync.dma_start(dbg_h[b], h_tiles[b][:])
                nc.sync.dma_start(dbg_a[b], a_tiles[b][:])
            if b + 1 < BPC:
                emit_gn_qkv(b + 1)

    if not nc.is_finalized():
        nc.finalize()
    return nc


def _prep_inputs(x, norm_w, norm_b, qkv_w, qkv_b, proj_w, proj_b):
    """Fold norms/biases/scales into weights; reshape for the kernel layout."""
    f = np.float32
    x = np.asarray(x, f)
    nw = np.asarray(norm_w, f)
    nb = np.asarray(norm_b, f)
    qkv_w = np.asarray(qkv_w, f)
    qkv_b = np.asarray(qkv_b, f)
    proj_w = np.asarray(proj_w, f)
    proj_b = np.asarray(proj_b, f)

    Wq, Wk, Wv = qkv_w[0:C], qkv_w[C:2 * C], qkv_w[2 * C:3 * C]
    bqv, bkv, bvv = qkv_b[0:C], qkv_b[C:2 * C], qkv_b[2 * C:3 * C]
    scale = f(1.0 / np.sqrt(CH))
    # fp8 scaling: q' = 8q, k' = 8k, v' = 8v; Wq gets an extra x8 (undone in
    # the q psum copy) so its fp8 entries sit in the normal range.
    Wq_e = (Wq * nw[None, :]) * scale * 64.0
    bq_e = (Wq @ nb + bqv) * scale * 8.0
    Wk_e = (Wk * nw[None, :]) * 8.0   # k bias dropped (softmax shift invariance)
    Wv_e = (Wv * nw[None, :]) * 8.0
    bv_e = Wv @ nb + bvv
    bp_e = proj_b + proj_w @ bv_e     # v bias folded into proj bias
    Wp_e = proj_w * 8.0

    def chan_chunks(vec):  # [C] -> [128, KO]
        return np.ascontiguousarray(vec.reshape(KO, 128).T)

    def lhsT_chunks(wT, dtype):  # [C, M] -> [128, KO, M]
        return np.ascontiguousarray(
            wT.reshape(KO, 128, wT.shape[1]).transpose(1, 0, 2)).astype(dtype)

    wqkT = np.concatenate([Wq_e, Wk_e], axis=0).T  # [C, 1024]
    gm = np.zeros((C, NG), f)
    gm[np.arange(C), np.arange(C) // (C // NG)] = 1.0 / (C // NG)
    bm = np.zeros((128, C), f)
    bm[np.arange(C) // (C // NG), np.arange(C)] = 1.0

    d8 = _np8()
    bpp = np.zeros((128, 2, C), np.float32)
    bpp[0, 1, :] = 64.0 * bp_e
    pat = np.zeros((128, 3 * T), d8)
    pat[0, 0:T] = d8.type(BETA)
    pat[0, T:2 * T] = d8.type(1.0)
    rbc = np.zeros((128, 2, 128), np.float32)
    for m in range(128):
        for half in range(2):
            rbc[2 * (m // 64) + half, half, m] = 1.0
    shared = {
        "pat": pat,
        "rbc": rbc.astype(_npbf()),
        "wqkT": lhsT_chunks(wqkT, d8),
        "wvT": lhsT_chunks(Wv_e.T, d8),
        "wpT": lhsT_chunks(Wp_e.T, d8),
        "bq": chan_chunks(bq_e),
        "bppat": bpp.astype(d8),
        "gmat": np.ascontiguousarray(
            gm.reshape(KO, 128, NG).transpose(1, 0, 2)),
        "bmat": np.ascontiguousarray(bm.reshape(128, KO, 128)),
        "ones": np.ones((128, 64), d8),
    }
    xr = x.reshape(B, C, T)
    in_maps = []
    for c in range(NCORES):
        xc = xr[c * BPC:(c + 1) * BPC].reshape(BPC, KO, 128, T).transpose(0, 2, 1, 3)
        m = dict(shared)
        m["x"] = np.ascontiguousarray(xc).astype(_npbf())
        in_maps.append(m)
    return in_maps


def kernel(x, norm_w, norm_b, qkv_w, qkv_b, proj_w, proj_b):
    from concourse.bass_utils import run_bass_kernel_spmd

    in_maps = _prep_inputs(x, norm_w, norm_b, qkv_w, qkv_b, proj_w, proj_b)
    nc = _build_nc()
    res = run_bass_kernel_spmd(nc, in_maps, core_ids=list(range(NCORES)), trace=TRACE)
    kernel.last_results = res
    outs = []
    for c in range(NCORES):
        oc = res.results[c]["out"]  # [BPC, 128, KO, T]
        outs.append(np.asarray(oc).transpose(0, 2, 1, 3).reshape(BPC, C, T))
    full = np.concatenate(outs, axis=0).reshape(B, C, 32, 32).astype(np.float32)
    return full
